# revision 11
# baseline (speedup 1.0000x reference)
"""Trainium2 Bass kernel for nn_AdaptiveFusionNet (8-core data parallel).

Math collapse (validated vs reference to ~5e-6 rel):
  - DCT branch: apply_dct(x)[b,c] == A @ X @ A.T with A = R @ D[:,:25] @ D[:25,:]
    (R = 256->8 bilinear resize matrix; note reference's "inverse" einsum
    applies D again, not D.T).
  - Gradient branch: bilinear 256->8 resize samples grad magnitude only at
    rows/cols {15,16}+32g with weight 1/4 each; sobel there needs only the
    4x4 neighborhoods {14..17}+32g (wrap never triggered). So only 32x32
    samples per channel are needed.
  - conv+BN+ReLU folded: w_eff = w*gamma/sqrt(1+eps), b_eff = b*g+beta.
  - Final: out = w*Pd + (1-w)*Pg + cls_b computed on host from per-core
    [8,10] device outputs ((Pd0,Pg0,Pd1,Pg1,slogit) x image-half).

Per core (16 images, 48 image-channels):
  loads (SP HWDGE + PE):  gp row-gather DMA (32 of 256 rows, ic-major) +
                per-image x DMAs feeding stage-1 matmuls
                w1t[cb][k, ic*128+b*8+i] = sum_r X[r, cb*128+k] * A[i, r]
                (192 f32 matmuls, lhsT = X blocks, N=8), packed 384 cols
                per PSUM bank.
  tail:         stage2 (PE) psum2[j, ic*128+b*8+i] = dct8[b,ic][i,j];
                transpose (PE) -> dt_sb -> 3 sbuf DMAs -> dconv interior
                (borders pre-zeroed ONCE; interior copies never touch
                them); sobel on the sampled 16x16 points (ACT col-sample
                + DVE); convs (PE f32r, 9-shift matmuls K=3, N=512, both
                branches stacked on PSUM partitions); relu (ACT, folded
                BN bias); reductions (DVE); 3 tiny matmuls -> out[8,10].
Final sigmoid-gated combine runs on host (cheap on [128,*]).

Timing-loop pipelining: For_i_pipelined([loads, tail], unroll=2,
staggered_reset) software-pipelines iterations: tick i's tail overlaps
tick i+1's loads. DMAs with late-arriving deps (dflat/gflat/out)
dispatch from the ACT HWDGE queue so the SP queue (x/gp loads) never
head-of-line blocks. PSUM: 8 banks = w1t x2cb x2ring + psc x2 +
(psum2|psumT merged) + po.
"""
import os
import sys

import numpy as np

try:
    import concourse  # noqa: F401
except ImportError:
    sys.path.insert(0, "/opt/trn_rl_repo")

import concourse.bass as bass
import concourse.bacc as bacc
import concourse.mybir as mybir
from concourse import tile
from concourse.bass_utils import run_bass_kernel_spmd

F32 = mybir.dt.float32
F32R = mybir.dt.float32r
N_CORES = 8
B_TOTAL = 128
B = B_TOTAL // N_CORES  # 16 images per core
NCH = B * 3             # 48 channels per core


def build_A():
    N = 256
    n = np.arange(N, dtype=np.float64)
    k = n[:, None]
    D = np.cos(np.pi * (2.0 * n + 1.0) * k / (2.0 * N))
    scale = np.full((N, 1), np.sqrt(2.0 / N))
    scale[0, 0] = np.sqrt(1.0 / N)
    D = D * scale
    R = np.zeros((8, 256))
    for i in range(8):
        R[i, 15 + 32 * i] = 0.5
        R[i, 16 + 32 * i] = 0.5
    A = R @ D[:, :25] @ D[:25, :]
    return A.astype(np.float32)


def _build_nc(timing_loop=None):
    drop = set(os.environ.get("KDROP", "").split(","))
    nc = bacc.Bacc("TRN2", target_bir_lowering=False, debug=False,
                   num_devices=N_CORES)

    if timing_loop is None:
        x_d = nc.dram_tensor("x", [B, 3, 256, 256], F32, kind="ExternalInput")
    else:
        x_d = nc.dram_tensor("xint", [B, 3, 256, 256], F32)
    at_d = nc.dram_tensor("at", [256, 8], F32, kind="ExternalInput")
    id_d = nc.dram_tensor("ident", [8, 8], F32, kind="ExternalInput")
    convw_d = nc.dram_tensor("convw", [3, 1152], F32R, kind="ExternalInput")
    bias_d = nc.dram_tensor("bias", [128, 1], F32, kind="ExternalInput")
    ccls_d = nc.dram_tensor("ccls", [128, 128], F32, kind="ExternalInput")
    fusw_d = nc.dram_tensor("fusw", [128, 1], F32, kind="ExternalInput")
    sel_d = nc.dram_tensor("sel", [128, 2], F32, kind="ExternalInput")
    out_d = nc.dram_tensor("out", [B, 5], F32, kind="ExternalOutput")

    SUB = mybir.AluOpType.subtract
    ADD = mybir.AluOpType.add
    MUL = mybir.AluOpType.mult
    RING = 2  # software-pipeline depth for loads->tail tiles

    with tile.TileContext(nc) as tc:
        with (
            tc.tile_pool(name="const", bufs=1) as cpool,
            tc.tile_pool(name="xin", bufs=6) as xpool,
            tc.tile_pool(name="work", bufs=2) as wpool,
            tc.tile_pool(name="scratch", bufs=2) as spool,
            tc.tile_pool(name="ps1", bufs=1, space="PSUM") as ps1,
            tc.tile_pool(name="ps2", bufs=1, space="PSUM") as ps2,
        ):
            # ---- constants ----
            a1t = cpool.tile([128, 8], F32, tag="a1t")
            a2t = cpool.tile([128, 8], F32, tag="a2t")
            nc.sync.dma_start(a1t[:], at_d[0:128, :])
            nc.sync.dma_start(a2t[:], at_d[128:256, :])
            ident = cpool.tile([8, 8], F32, tag="ident")
            nc.sync.dma_start(ident[:], id_d[:])
            convw = cpool.tile([3, 1152], F32R, tag="convw")
            nc.sync.dma_start(convw[:], convw_d[:])
            biasd = cpool.tile([64, 1], F32, tag="biasd")
            biasg = cpool.tile([64, 1], F32, tag="biasg")
            nc.sync.dma_start(biasd[:], bias_d[0:64, :])
            nc.sync.dma_start(biasg[:], bias_d[64:128, :])
            zpad = cpool.tile([3, 160], F32, tag="zpad")
            nc.vector.memset(zpad[:], 0.0)
            ccls = cpool.tile([128, 128], F32, tag="ccls")
            nc.sync.dma_start(ccls[:], ccls_d[:])
            fusw = cpool.tile([128, 1], F32, tag="fusw")
            nc.sync.dma_start(fusw[:], fusw_d[:])
            sel = cpool.tile([128, 2], F32, tag="sel")
            nc.sync.dma_start(sel[:], sel_d[:])

            # conv-input ring tiles live across iterations; borders zeroed
            # once, per-iteration copies only write the 8x8 interior.
            dconv_ring = [cpool.tile([3, B, 10, 10], F32R, tag=f"dconv{i}",
                                     name=f"dconv{i}") for i in range(RING)]
            gconv_ring = [cpool.tile([3, B, 10, 10], F32R, tag=f"gconv{i}",
                                     name=f"gconv{i}") for i in range(RING)]

            def zero_borders(pad_t):
                z10 = zpad.rearrange("p (b w) -> p b w", b=B)           # [3,16,10]
                z8 = zpad[:, 0:128].rearrange("p (b w) -> p b w", b=B)  # [3,16,8]
                nc.vector.tensor_copy(pad_t[:, :, 0, :], z10)
                nc.vector.tensor_copy(pad_t[:, :, 9, :], z10)
                nc.vector.tensor_copy(pad_t[:, :, 1:9, 0], z8)
                nc.vector.tensor_copy(pad_t[:, :, 1:9, 9], z8)

            for t in dconv_ring + gconv_ring:
                zero_borders(t)

            # ring buffers handed from loads to mid (software pipeline)
            gp_ring = [wpool.tile([NCH, 8, 4, 256], F32, tag=f"gp{i}",
                                  bufs=1, name=f"gp{i}") for i in range(RING)]
            w1t_single = [ps1.tile([128, 384], F32, tag=f"w1t{cb}",
                                   name=f"w1t{cb}") for cb in range(2)]

            def do_loads(gp, w1t):
                """DMA x in + stage-1 matmuls. SP HWDGE queue only (every
                DMA with a late-arriving dep is on the ACT queue), so SP
                never head-of-line blocks the next iteration's loads."""
                if "gpdma" not in drop:
                    for ic in range(3):
                        src = x_d[:, ic].rearrange(
                            "b (g h) w -> b g h w", h=32)[:, :, 14:18, :]
                        nc.sync.dma_start(gp[ic * B:(ic + 1) * B], src)

                for b in range(B):
                    xt = xpool.tile([128, 3, 2, 256], F32, tag="xt")
                    nc.sync.dma_start(
                        xt[:], x_d[b].rearrange("c (rb p) w -> p c rb w", p=128))
                    for ic in range(3):
                        for cb in range(2):
                            for rb in range(2):
                                nc.tensor.matmul(
                                    w1t[cb][:, ic * 128 + b * 8:
                                            ic * 128 + (b + 1) * 8],
                                    lhsT=xt[:, ic, rb, cb * 128:(cb + 1) * 128],
                                    rhs=(a1t[:] if rb == 0 else a2t[:]),
                                    start=(rb == 0), stop=(rb == 1))

            def do_mid(gp, w1t, dconv, gconv):
                # ---- stage 2: dct8^T ----
                dcttail = "dcttail" not in drop
                w1sb = [wpool.tile([128, 384], F32, tag=f"w1sb{cb}",
                                   name=f"w1sb{cb}") for cb in range(2)]
                if dcttail:
                    for cb in range(2):
                        nc.vector.tensor_copy(w1sb[cb][:], w1t[cb][:])
                ps2m = ps2.tile([128, 512], F32, tag="ps2m", name="ps2m")
                psum2 = ps2m[0:8, 0:384]
                psumT = ps2m[:, 384:408]
                if dcttail:
                    for ic in range(3):
                        for cb in range(2):
                            nc.tensor.matmul(
                                psum2[:, ic * 128:(ic + 1) * 128],
                                lhsT=(a1t[:] if cb == 0 else a2t[:]),
                                rhs=w1sb[cb][:, ic * 128:(ic + 1) * 128],
                                start=(cb == 0), stop=(cb == 1))

                dsb = wpool.tile([8, 384], F32, tag="dsb")
                dt_sb = wpool.tile([128, 24], F32, tag="dt_sb")
                dflat = wpool.tile([3, 1024], F32, tag="dflat")

                if dcttail:
                    nc.vector.tensor_copy(dsb[:], psum2)
                    for ic in range(3):
                        nc.tensor.transpose(
                            psumT[:, ic * 8:(ic + 1) * 8],
                            dsb[0:8, ic * 128:(ic + 1) * 128],
                            ident[:])
                    nc.vector.tensor_copy(dt_sb[:], psumT)
                    for ic in range(3):
                        nc.scalar.dma_start(dflat[ic:ic + 1, :],
                                            dt_sb[:, ic * 8:(ic + 1) * 8])
                    nc.vector.tensor_copy(
                        dconv[:, :, 1:9, 1:9],
                        dflat.rearrange("c (b i j) -> c b i j", b=B, i=8))

                # ---- gradient branch: col sample + sobel ----
                gpatch = wpool.tile([NCH, 8, 4, 8, 4], F32, tag="gpatch")
                g8 = wpool.tile([NCH, 8, 8, 1], F32, tag="g8")
                gflat = wpool.tile([3, 1024], F32, tag="gflat")
                if "sobel" not in drop:
                    gp5 = gp.rearrange("p g r (gc c) -> p g r gc c", c=32)
                    for g in range(8):
                        nc.scalar.copy(gpatch[:, g], gp5[:, g, :, :, 14:18])

                    def PP(r, dc):
                        return gpatch[:, :, r, :, 1 + dc:3 + dc]  # [48, 8, 8, 2]

                    a_lr = []
                    for lr in (1, 2):
                        t1 = spool.tile([NCH, 8, 8, 2], F32, tag="t1")
                        t2 = spool.tile([NCH, 8, 8, 2], F32, tag="t2")
                        t3 = spool.tile([NCH, 8, 8, 2], F32, tag="t3")
                        nc.vector.tensor_tensor(t1[:], PP(lr - 1, 1), PP(lr - 1, -1), SUB)
                        nc.vector.tensor_tensor(t2[:], PP(lr, 1), PP(lr, -1), SUB)
                        nc.vector.tensor_tensor(t3[:], PP(lr + 1, 1), PP(lr + 1, -1), SUB)
                        u = spool.tile([NCH, 8, 8, 2], F32, tag="u")
                        nc.vector.tensor_tensor(u[:], t1[:], t3[:], ADD)
                        gx = spool.tile([NCH, 8, 8, 2], F32, tag="gx")
                        nc.vector.scalar_tensor_tensor(gx[:], t2[:], 2.0, u[:], MUL, ADD)
                        s1 = spool.tile([NCH, 8, 8, 2], F32, tag="s1")
                        s2 = spool.tile([NCH, 8, 8, 2], F32, tag="s2")
                        s3 = spool.tile([NCH, 8, 8, 2], F32, tag="s3")
                        nc.vector.tensor_tensor(s1[:], PP(lr + 1, -1), PP(lr - 1, -1), SUB)
                        nc.vector.tensor_tensor(s2[:], PP(lr + 1, 0), PP(lr - 1, 0), SUB)
                        nc.vector.tensor_tensor(s3[:], PP(lr + 1, 1), PP(lr - 1, 1), SUB)
                        u2 = spool.tile([NCH, 8, 8, 2], F32, tag="u2")
                        nc.vector.tensor_tensor(u2[:], s1[:], s3[:], ADD)
                        gy = spool.tile([NCH, 8, 8, 2], F32, tag="gy")
                        nc.vector.scalar_tensor_tensor(gy[:], s2[:], 2.0, u2[:], MUL, ADD)
                        nc.vector.tensor_tensor(gx[:], gx[:], gx[:], MUL)
                        nc.vector.tensor_tensor(gy[:], gy[:], gy[:], MUL)
                        m2 = spool.tile([NCH, 8, 8, 2], F32, tag="m2")
                        nc.vector.tensor_tensor(m2[:], gx[:], gy[:], ADD)
                        mag = spool.tile([NCH, 8, 8, 2], F32, tag=f"mag{lr}")
                        nc.scalar.sqrt(mag[:], m2[:])
                        al = spool.tile([NCH, 8, 8, 1], F32, tag=f"al{lr}")
                        nc.vector.tensor_tensor(al[:], mag[:, :, :, 0:1], mag[:, :, :, 1:2], ADD)
                        a_lr.append(al)

                    nc.vector.tensor_tensor(g8[:], a_lr[0][:], a_lr[1][:], ADD)
                    g8f = g8.rearrange("p a b c -> p (a b c)")  # [48, 64]

                    for ic in range(3):
                        nc.scalar.dma_start(gflat[ic:ic + 1, :],
                                            g8f[ic * B:(ic + 1) * B, :])
                    nc.vector.tensor_copy(
                        gconv[:, :, 1:9, 1:9],
                        gflat.rearrange("c (b i j) -> c b i j", b=B, i=8))

            def do_late(dconv, gconv):
                # ---- convs (f32r) ----
                psc = [[ps1.tile([64, 512], F32, tag=f"psc{br}{nh}",
                                 name=f"psc{br}{nh}") for nh in range(2)]
                       for br in range(2)]
                if not ("conv" in drop or "tail" in drop):
                    for nh in range(2):
                        for br, rhs_t in ((0, dconv), (1, gconv)):
                            for si, (di, dj) in enumerate(
                                    (di, dj) for di in range(3) for dj in range(3)):
                                w_off = (br * 9 + di * 3 + dj) * 64
                                rv = rhs_t[:, nh * 8:(nh + 1) * 8,
                                           di:di + 8, dj:dj + 8]
                                nc.tensor.matmul(
                                    psc[br][nh][:, :],
                                    lhsT=convw[:, w_off:w_off + 64],
                                    rhs=rv,
                                    start=(si == 0), stop=(si == 8))

                # relu(conv + bias) -> dg_sb [128, 1024]
                tail_on = "tail" not in drop
                dg_sb = wpool.tile([128, 1024], F32, tag="dg_sb")
                po = ps2.tile([8, 10], F32, tag="psO", name="po")
                if tail_on:
                    for nh in range(2):
                        for br in range(2):
                            nc.scalar.activation(
                                dg_sb[br * 64:(br + 1) * 64,
                                      nh * 512:(nh + 1) * 512],
                                psc[br][nh][:],
                                mybir.ActivationFunctionType.Relu,
                                bias=(biasd[:] if br == 0 else biasg[:]),
                                scale=1.0)

                    # ---- fusion + classifier (per half for pipelining) ----
                    HB = B // 2
                    for nh in range(2):
                        dgh = dg_sb[:, nh * 512:(nh + 1) * 512].rearrange(
                            "p (b f) -> p b f", b=HB)
                        s_red = wpool.tile([128, HB, 1], F32, tag=f"s_red{nh}",
                                           name=f"s_red{nh}")
                        nc.vector.reduce_sum(s_red[:], dgh,
                                             axis=mybir.AxisListType.X)
                        tk_red = []
                        for k in range(2):
                            tmpk = spool.tile([128, HB, 64], F32, tag="tmpk")
                            cc = ccls[:, k * 64:(k + 1) * 64].unsqueeze(1)
                            nc.gpsimd.tensor_tensor(
                                tmpk[:], dgh, cc.broadcast_to([128, HB, 64]), MUL)
                            tkr = wpool.tile([128, HB, 1], F32,
                                             tag=f"tkr_{nh}_{k}",
                                             name=f"tkr_{nh}_{k}")
                            nc.vector.reduce_sum(tkr[:], tmpk[:],
                                                 axis=mybir.AxisListType.X)
                            tk_red.append(tkr)
                        for k in range(2):
                            nc.tensor.matmul(
                                po[0:HB, nh * 5 + 2 * k:nh * 5 + 2 * k + 2],
                                lhsT=tk_red[k][:], rhs=sel[:],
                                start=True, stop=True)
                        nc.tensor.matmul(po[0:HB, nh * 5 + 4:nh * 5 + 5],
                                         lhsT=s_red[:], rhs=fusw[:],
                                         start=True, stop=True)

                    osb = wpool.tile([8, 10], F32, tag="osb")
                    nc.scalar.copy(osb[:], po[:])
                    nc.scalar.dma_start(
                        out_d.rearrange("(nh b) c -> b nh c", nh=2),
                        osb.rearrange("b (nh c) -> b nh c", nh=2))
                else:
                    out_sb = wpool.tile([16, 5], F32, tag="out_sb")
                    nc.vector.memset(out_sb[:], 0.0)
                    nc.scalar.dma_start(out_d[:], out_sb[:])

            if timing_loop is None:
                do_loads(gp_ring[0], w1t_single)
                do_mid(gp_ring[0], w1t_single, dconv_ring[0], gconv_ring[0])
                do_late(dconv_ring[0], gconv_ring[0])
            else:
                # 3-stage software pipeline over the timing loop: tick i's
                # convs/output (late) + tick i+1's dct/sobel tail (mid) +
                # tick i+2's loads all overlap, so the in-order PE stream
                # [late, mid-PE, stage-1] never stalls on the sobel chain
                # and the x DMA stream stays saturated. One tick == one
                # full kernel evaluation.
                hints = (mybir.EngineType.PE,
                         mybir.EngineType.SP,
                         mybir.EngineType.DVE,
                         mybir.EngineType.Activation)

                def stage_loads(pipe, iv):
                    gp = pipe.intermediate_tile(
                        [NCH, 8, 4, 256], F32, name="gp", prealloc=gp_ring)
                    w1t0 = pipe.intermediate_tile(
                        [128, 384], F32, name="w1t0", bufs=1,
                        prealloc=[w1t_single[0]])
                    w1t1 = pipe.intermediate_tile(
                        [128, 384], F32, name="w1t1", bufs=1,
                        prealloc=[w1t_single[1]])
                    do_loads(gp, [w1t0, w1t1])
                    return (gp, w1t0, w1t1)

                def stage_mid(pipe, iv, tl):
                    gp, w1t0, w1t1 = tl
                    dconv = pipe.intermediate_tile(
                        [3, B, 10, 10], F32R, name="dconv",
                        prealloc=dconv_ring)
                    gconv = pipe.intermediate_tile(
                        [3, B, 10, 10], F32R, name="gconv",
                        prealloc=gconv_ring)
                    do_mid(gp, [w1t0, w1t1], dconv, gconv)
                    return (dconv, gconv)

                def stage_late(pipe, iv, tl):
                    dconv, gconv = tl
                    do_late(dconv, gconv)

                tc.For_i_pipelined([stage_loads, stage_mid, stage_late],
                                   0, timing_loop,
                                   unroll=4, staged_num_bufs=RING,
                                   staggered_reset=True, auto_markers=hints,
                                   hint_engines=hints, name="mainloop")

    nc.compile()
    return nc


_NC = {}


def _get_nc(timing_loop=None):
    if timing_loop not in _NC:
        _NC[timing_loop] = _build_nc(timing_loop)
    return _NC[timing_loop]


def _make_consts(conv_dct_w, conv_dct_b, bn_dct_g, bn_dct_b,
                 conv_grad_w, conv_grad_b, bn_grad_g, bn_grad_b,
                 fus_w, cls_w):
    A = build_A()
    consts = {}
    consts["at"] = np.ascontiguousarray(A.T)
    consts["ident"] = np.eye(8, dtype=np.float32)

    BN_EPS = 1e-5
    convw = np.zeros((3, 1152), np.float32)
    bias = np.zeros((128, 1), np.float32)
    for br, (w, b, g, beta) in enumerate((
            (conv_dct_w, conv_dct_b, bn_dct_g, bn_dct_b),
            (conv_grad_w, conv_grad_b, bn_grad_g, bn_grad_b))):
        g_eff = (g / np.sqrt(1.0 + BN_EPS)).astype(np.float32)
        w_eff = w * g_eff[:, None, None, None]
        if br == 1:
            w_eff = w_eff * 0.25  # fold the 4-sample average
        b_eff = b * g_eff + beta
        for di in range(3):
            for dj in range(3):
                # convw[ic, (br*9+di*3+dj)*64 + oc] = w_eff[oc, ic, di, dj]
                off = (br * 9 + di * 3 + dj) * 64
                convw[:, off:off + 64] = w_eff[:, :, di, dj].T
        bias[br * 64:(br + 1) * 64, 0] = b_eff
    consts["convw"] = convw
    consts["bias"] = bias

    ccls = np.zeros((128, 128), np.float32)
    for k in range(2):
        ccls[0:64, k * 64:(k + 1) * 64] = cls_w[k].reshape(64, 64)
        ccls[64:128, k * 64:(k + 1) * 64] = cls_w[k].reshape(64, 64)
    consts["ccls"] = ccls
    consts["fusw"] = np.ascontiguousarray(np.tile(fus_w[0][:, None] / 64.0, (2, 1)))
    sel = np.zeros((128, 2), np.float32)
    sel[0:64, 0] = 1.0
    sel[64:128, 1] = 1.0
    consts["sel"] = sel
    return consts


def kernel_with_results(x, conv_dct_w, conv_dct_b, bn_dct_g, bn_dct_b,
                        conv_grad_w, conv_grad_b, bn_grad_g, bn_grad_b,
                        fus_w, fus_b, cls_w, cls_b, trace=False):
    nc = _get_nc()
    consts = _make_consts(conv_dct_w, conv_dct_b, bn_dct_g, bn_dct_b,
                          conv_grad_w, conv_grad_b, bn_grad_g, bn_grad_b,
                          fus_w, cls_w)
    x = np.ascontiguousarray(np.asarray(x, np.float32))
    in_maps = []
    for i in range(N_CORES):
        m = {"x": np.ascontiguousarray(x[i * B:(i + 1) * B])}
        m.update(consts)
        in_maps.append(m)
    res = run_bass_kernel_spmd(nc, in_maps, list(range(N_CORES)), trace=trace)

    outs = []
    for i in range(N_CORES):
        r = res.results[i]["out"]  # [16, 5]
        Pd = r[:, [0, 2]]
        Pg = r[:, [1, 3]]
        sl = r[:, 4] + np.float32(fus_b[0])
        w = 1.0 / (1.0 + np.exp(-sl))[:, None]
        outs.append(w * Pd + (1.0 - w) * Pg + np.asarray(cls_b)[None, :])
    return np.concatenate(outs, axis=0).astype(np.float32), res


def kernel(**inputs):
    out, _ = kernel_with_results(**inputs)
    return out


# revision 12
# speedup vs baseline: 1.7513x; 1.7513x over previous
"""Trainium2 Bass kernel for nn_AdaptiveFusionNet (8-core data parallel).

Math collapse (validated vs reference to ~5e-6 rel):
  - DCT branch: apply_dct(x)[b,c] == A @ X @ A.T with A = R @ D[:,:25] @ D[:25,:]
    (R = 256->8 bilinear resize matrix; note reference's "inverse" einsum
    applies D again, not D.T).
  - Gradient branch: bilinear 256->8 resize samples grad magnitude only at
    rows/cols {15,16}+32g with weight 1/4 each; sobel there needs only the
    4x4 neighborhoods {14..17}+32g (wrap never triggered). So only 32x32
    samples per channel are needed.
  - conv+BN+ReLU folded: w_eff = w*gamma/sqrt(1+eps), b_eff = b*g+beta.
  - Final: out = w*Pd + (1-w)*Pg + cls_b computed on host from per-core
    [8,10] device outputs ((Pd0,Pg0,Pd1,Pg1,slogit) x image-half).

Per core (16 images, 48 image-channels):
  loads (SP HWDGE + PE):  gp row-gather DMA (32 of 256 rows, ic-major) +
                per-image x DMAs feeding stage-1 matmuls
                w1t[cb][k, ic*128+b*8+i] = sum_r X[r, cb*128+k] * A[i, r]
                (192 f32 matmuls, lhsT = X blocks, N=8), packed 384 cols
                per PSUM bank.
  tail:         stage2 (PE) psum2[j, ic*128+b*8+i] = dct8[b,ic][i,j];
                transpose (PE) -> dt_sb -> 3 sbuf DMAs -> dconv interior
                (borders pre-zeroed ONCE; interior copies never touch
                them); sobel on the sampled 16x16 points (ACT col-sample
                + DVE); convs (PE f32r, 9-shift matmuls K=3, N=512, both
                branches stacked on PSUM partitions); relu (ACT, folded
                BN bias); reductions (DVE); 3 tiny matmuls -> out[8,10].
Final sigmoid-gated combine runs on host (cheap on [128,*]).

Timing-loop pipelining: For_i_pipelined([loads, tail], unroll=2,
staggered_reset) software-pipelines iterations: tick i's tail overlaps
tick i+1's loads. DMAs with late-arriving deps (dflat/gflat/out)
dispatch from the ACT HWDGE queue so the SP queue (x/gp loads) never
head-of-line blocks. PSUM: 8 banks = w1t x2cb x2ring + psc x2 +
(psum2|psumT merged) + po.
"""
import os
import sys

import numpy as np

try:
    import concourse  # noqa: F401
except ImportError:
    sys.path.insert(0, "/opt/trn_rl_repo")

import concourse.bass as bass
import concourse.bacc as bacc
import concourse.mybir as mybir
from concourse import tile
from concourse.bass_utils import run_bass_kernel_spmd

F32 = mybir.dt.float32
F32R = mybir.dt.float32r
N_CORES = 8
B_TOTAL = 128
B = B_TOTAL // N_CORES  # 16 images per core
NCH = B * 3             # 48 channels per core


def build_A():
    N = 256
    n = np.arange(N, dtype=np.float64)
    k = n[:, None]
    D = np.cos(np.pi * (2.0 * n + 1.0) * k / (2.0 * N))
    scale = np.full((N, 1), np.sqrt(2.0 / N))
    scale[0, 0] = np.sqrt(1.0 / N)
    D = D * scale
    R = np.zeros((8, 256))
    for i in range(8):
        R[i, 15 + 32 * i] = 0.5
        R[i, 16 + 32 * i] = 0.5
    A = R @ D[:, :25] @ D[:25, :]
    return A.astype(np.float32)


def _build_nc(timing_loop=None):
    drop = set(os.environ.get("KDROP", "").split(","))
    nc = bacc.Bacc("TRN2", target_bir_lowering=False, debug=False,
                   num_devices=N_CORES)

    if timing_loop is None:
        x_d = nc.dram_tensor("x", [B, 3, 256, 256], F32, kind="ExternalInput")
    else:
        x_d = nc.dram_tensor("xint", [B, 3, 256, 256], F32)
    at_d = nc.dram_tensor("at", [256, 8], F32, kind="ExternalInput")
    id_d = nc.dram_tensor("ident", [8, 8], F32, kind="ExternalInput")
    convw_d = nc.dram_tensor("convw", [3, 1152], F32R, kind="ExternalInput")
    bias_d = nc.dram_tensor("bias", [128, 1], F32, kind="ExternalInput")
    ccls_d = nc.dram_tensor("ccls", [128, 128], F32, kind="ExternalInput")
    fusw_d = nc.dram_tensor("fusw", [128, 1], F32, kind="ExternalInput")
    sel_d = nc.dram_tensor("sel", [128, 2], F32, kind="ExternalInput")
    out_d = nc.dram_tensor("out", [B, 5], F32, kind="ExternalOutput")

    SUB = mybir.AluOpType.subtract
    ADD = mybir.AluOpType.add
    MUL = mybir.AluOpType.mult
    RING = 2  # software-pipeline depth for loads->tail tiles

    with tile.TileContext(nc) as tc:
        with (
            tc.tile_pool(name="const", bufs=1) as cpool,
            tc.tile_pool(name="xin", bufs=6) as xpool,
            tc.tile_pool(name="work", bufs=2) as wpool,
            tc.tile_pool(name="scratch", bufs=2) as spool,
            tc.tile_pool(name="ps1", bufs=1, space="PSUM") as ps1,
            tc.tile_pool(name="ps2", bufs=1, space="PSUM") as ps2,
        ):
            # ---- constants ----
            a1t = cpool.tile([128, 8], F32, tag="a1t")
            a2t = cpool.tile([128, 8], F32, tag="a2t")
            nc.sync.dma_start(a1t[:], at_d[0:128, :])
            nc.sync.dma_start(a2t[:], at_d[128:256, :])
            ident = cpool.tile([8, 8], F32, tag="ident")
            nc.sync.dma_start(ident[:], id_d[:])
            convw = cpool.tile([3, 1152], F32R, tag="convw")
            nc.sync.dma_start(convw[:], convw_d[:])
            biasd = cpool.tile([64, 1], F32, tag="biasd")
            biasg = cpool.tile([64, 1], F32, tag="biasg")
            nc.sync.dma_start(biasd[:], bias_d[0:64, :])
            nc.sync.dma_start(biasg[:], bias_d[64:128, :])
            zpad = cpool.tile([3, 160], F32, tag="zpad")
            nc.vector.memset(zpad[:], 0.0)
            ccls = cpool.tile([128, 128], F32, tag="ccls")
            nc.sync.dma_start(ccls[:], ccls_d[:])
            fusw = cpool.tile([128, 1], F32, tag="fusw")
            nc.sync.dma_start(fusw[:], fusw_d[:])
            sel = cpool.tile([128, 2], F32, tag="sel")
            nc.sync.dma_start(sel[:], sel_d[:])

            # conv-input ring tiles live across iterations; borders zeroed
            # once, per-iteration copies only write the 8x8 interior.
            dconv_ring = [cpool.tile([3, B, 10, 10], F32R, tag=f"dconv{i}",
                                     name=f"dconv{i}") for i in range(RING)]
            gconv_ring = [cpool.tile([3, B, 10, 10], F32R, tag=f"gconv{i}",
                                     name=f"gconv{i}") for i in range(RING)]

            def zero_borders(pad_t):
                z10 = zpad.rearrange("p (b w) -> p b w", b=B)           # [3,16,10]
                z8 = zpad[:, 0:128].rearrange("p (b w) -> p b w", b=B)  # [3,16,8]
                nc.vector.tensor_copy(pad_t[:, :, 0, :], z10)
                nc.vector.tensor_copy(pad_t[:, :, 9, :], z10)
                nc.vector.tensor_copy(pad_t[:, :, 1:9, 0], z8)
                nc.vector.tensor_copy(pad_t[:, :, 1:9, 9], z8)

            for t in dconv_ring + gconv_ring:
                zero_borders(t)

            # ring buffers handed from loads to mid (software pipeline)
            gp_ring = [wpool.tile([NCH, 8, 4, 256], F32, tag=f"gp{i}",
                                  bufs=1, name=f"gp{i}") for i in range(RING)]
            w1t_single = [ps1.tile([128, 384], F32, tag=f"w1t{cb}",
                                   name=f"w1t{cb}") for cb in range(2)]

            def do_loads(gp, w1t):
                """DMA x in + stage-1 matmuls. SP HWDGE queue only (every
                DMA with a late-arriving dep is on the ACT queue), so SP
                never head-of-line blocks the next iteration's loads."""
                if "gpdma" not in drop:
                    for ic in range(3):
                        src = x_d[:, ic].rearrange(
                            "b (g h) w -> b g h w", h=32)[:, :, 14:18, :]
                        nc.sync.dma_start(gp[ic * B:(ic + 1) * B], src)

                for b in range(B):
                    xt = xpool.tile([128, 3, 2, 256], F32, tag="xt")
                    nc.sync.dma_start(
                        xt[:], x_d[b].rearrange("c (rb p) w -> p c rb w", p=128))
                    for ic in range(3):
                        for cb in range(2):
                            for rb in range(2):
                                nc.tensor.matmul(
                                    w1t[cb][:, ic * 128 + b * 8:
                                            ic * 128 + (b + 1) * 8],
                                    lhsT=xt[:, ic, rb, cb * 128:(cb + 1) * 128],
                                    rhs=(a1t[:] if rb == 0 else a2t[:]),
                                    start=(rb == 0), stop=(rb == 1))

            def do_mid(gp, w1t, dconv, gconv):
                # ---- stage 2: dct8^T ----
                dcttail = "dcttail" not in drop
                w1sb = [wpool.tile([128, 384], F32, tag=f"w1sb{cb}",
                                   name=f"w1sb{cb}") for cb in range(2)]
                if dcttail:
                    for cb in range(2):
                        nc.vector.tensor_copy(w1sb[cb][:], w1t[cb][:])
                ps2m = ps2.tile([128, 512], F32, tag="ps2m", name="ps2m")
                psum2 = ps2m[0:8, 0:384]
                psumT = ps2m[:, 384:408]
                if dcttail:
                    for ic in range(3):
                        for cb in range(2):
                            nc.tensor.matmul(
                                psum2[:, ic * 128:(ic + 1) * 128],
                                lhsT=(a1t[:] if cb == 0 else a2t[:]),
                                rhs=w1sb[cb][:, ic * 128:(ic + 1) * 128],
                                start=(cb == 0), stop=(cb == 1))

                dsb = wpool.tile([8, 384], F32, tag="dsb")
                dt_sb = wpool.tile([128, 24], F32, tag="dt_sb")
                dflat = wpool.tile([3, 1024], F32, tag="dflat")

                if dcttail:
                    nc.vector.tensor_copy(dsb[:], psum2)
                    for ic in range(3):
                        nc.tensor.transpose(
                            psumT[:, ic * 8:(ic + 1) * 8],
                            dsb[0:8, ic * 128:(ic + 1) * 128],
                            ident[:])
                    nc.vector.tensor_copy(dt_sb[:], psumT)
                    for ic in range(3):
                        nc.scalar.dma_start(dflat[ic:ic + 1, :],
                                            dt_sb[:, ic * 8:(ic + 1) * 8])
                    nc.vector.tensor_copy(
                        dconv[:, :, 1:9, 1:9],
                        dflat.rearrange("c (b i j) -> c b i j", b=B, i=8))

                # ---- gradient branch: col sample + sobel ----
                gpatch = wpool.tile([NCH, 8, 4, 8, 4], F32, tag="gpatch")
                g8 = wpool.tile([NCH, 8, 8, 1], F32, tag="g8")
                gflat = wpool.tile([3, 1024], F32, tag="gflat")
                if "sobel" not in drop:
                    gp5 = gp.rearrange("p g r (gc c) -> p g r gc c", c=32)
                    for g in range(8):
                        nc.scalar.copy(gpatch[:, g], gp5[:, g, :, :, 14:18])

                    def PP(r, dc):
                        return gpatch[:, :, r, :, 1 + dc:3 + dc]  # [48, 8, 8, 2]

                    a_lr = []
                    for lr in (1, 2):
                        t1 = spool.tile([NCH, 8, 8, 2], F32, tag="t1")
                        t2 = spool.tile([NCH, 8, 8, 2], F32, tag="t2")
                        t3 = spool.tile([NCH, 8, 8, 2], F32, tag="t3")
                        nc.vector.tensor_tensor(t1[:], PP(lr - 1, 1), PP(lr - 1, -1), SUB)
                        nc.vector.tensor_tensor(t2[:], PP(lr, 1), PP(lr, -1), SUB)
                        nc.vector.tensor_tensor(t3[:], PP(lr + 1, 1), PP(lr + 1, -1), SUB)
                        u = spool.tile([NCH, 8, 8, 2], F32, tag="u")
                        nc.vector.tensor_tensor(u[:], t1[:], t3[:], ADD)
                        gx = spool.tile([NCH, 8, 8, 2], F32, tag="gx")
                        nc.vector.scalar_tensor_tensor(gx[:], t2[:], 2.0, u[:], MUL, ADD)
                        s1 = spool.tile([NCH, 8, 8, 2], F32, tag="s1")
                        s2 = spool.tile([NCH, 8, 8, 2], F32, tag="s2")
                        s3 = spool.tile([NCH, 8, 8, 2], F32, tag="s3")
                        nc.vector.tensor_tensor(s1[:], PP(lr + 1, -1), PP(lr - 1, -1), SUB)
                        nc.vector.tensor_tensor(s2[:], PP(lr + 1, 0), PP(lr - 1, 0), SUB)
                        nc.vector.tensor_tensor(s3[:], PP(lr + 1, 1), PP(lr - 1, 1), SUB)
                        u2 = spool.tile([NCH, 8, 8, 2], F32, tag="u2")
                        nc.vector.tensor_tensor(u2[:], s1[:], s3[:], ADD)
                        gy = spool.tile([NCH, 8, 8, 2], F32, tag="gy")
                        nc.vector.scalar_tensor_tensor(gy[:], s2[:], 2.0, u2[:], MUL, ADD)
                        nc.vector.tensor_tensor(gx[:], gx[:], gx[:], MUL)
                        nc.vector.tensor_tensor(gy[:], gy[:], gy[:], MUL)
                        m2 = spool.tile([NCH, 8, 8, 2], F32, tag="m2")
                        nc.vector.tensor_tensor(m2[:], gx[:], gy[:], ADD)
                        mag = spool.tile([NCH, 8, 8, 2], F32, tag=f"mag{lr}")
                        nc.scalar.sqrt(mag[:], m2[:])
                        al = spool.tile([NCH, 8, 8, 1], F32, tag=f"al{lr}")
                        nc.vector.tensor_tensor(al[:], mag[:, :, :, 0:1], mag[:, :, :, 1:2], ADD)
                        a_lr.append(al)

                    nc.vector.tensor_tensor(g8[:], a_lr[0][:], a_lr[1][:], ADD)
                    g8f = g8.rearrange("p a b c -> p (a b c)")  # [48, 64]

                    for ic in range(3):
                        nc.scalar.dma_start(gflat[ic:ic + 1, :],
                                            g8f[ic * B:(ic + 1) * B, :])
                    nc.vector.tensor_copy(
                        gconv[:, :, 1:9, 1:9],
                        gflat.rearrange("c (b i j) -> c b i j", b=B, i=8))

            def do_late(dconv, gconv):
                # ---- convs (f32r) ----
                psc = [[ps1.tile([64, 512], F32, tag=f"psc{br}{nh}",
                                 name=f"psc{br}{nh}") for nh in range(2)]
                       for br in range(2)]
                if not ("conv" in drop or "tail" in drop):
                    for nh in range(2):
                        for br, rhs_t in ((0, dconv), (1, gconv)):
                            for si, (di, dj) in enumerate(
                                    (di, dj) for di in range(3) for dj in range(3)):
                                w_off = (br * 9 + di * 3 + dj) * 64
                                rv = rhs_t[:, nh * 8:(nh + 1) * 8,
                                           di:di + 8, dj:dj + 8]
                                nc.tensor.matmul(
                                    psc[br][nh][:, :],
                                    lhsT=convw[:, w_off:w_off + 64],
                                    rhs=rv,
                                    start=(si == 0), stop=(si == 8))

                # relu(conv + bias) -> dg_sb [128, 1024]
                tail_on = "tail" not in drop
                dg_sb = wpool.tile([128, 1024], F32, tag="dg_sb")
                po = ps2.tile([8, 10], F32, tag="psO", name="po")
                if tail_on:
                    for nh in range(2):
                        for br in range(2):
                            nc.scalar.activation(
                                dg_sb[br * 64:(br + 1) * 64,
                                      nh * 512:(nh + 1) * 512],
                                psc[br][nh][:],
                                mybir.ActivationFunctionType.Relu,
                                bias=(biasd[:] if br == 0 else biasg[:]),
                                scale=1.0)

                    # ---- fusion + classifier (per half for pipelining) ----
                    HB = B // 2
                    for nh in range(2):
                        dgh = dg_sb[:, nh * 512:(nh + 1) * 512].rearrange(
                            "p (b f) -> p b f", b=HB)
                        s_red = wpool.tile([128, HB, 1], F32, tag=f"s_red{nh}",
                                           name=f"s_red{nh}")
                        nc.vector.reduce_sum(s_red[:], dgh,
                                             axis=mybir.AxisListType.X)
                        tk_red = []
                        for k in range(2):
                            tmpk = spool.tile([128, HB, 64], F32, tag="tmpk")
                            cc = ccls[:, k * 64:(k + 1) * 64].unsqueeze(1)
                            nc.gpsimd.tensor_tensor(
                                tmpk[:], dgh, cc.broadcast_to([128, HB, 64]), MUL)
                            tkr = wpool.tile([128, HB, 1], F32,
                                             tag=f"tkr_{nh}_{k}",
                                             name=f"tkr_{nh}_{k}")
                            nc.vector.reduce_sum(tkr[:], tmpk[:],
                                                 axis=mybir.AxisListType.X)
                            tk_red.append(tkr)
                        for k in range(2):
                            nc.tensor.matmul(
                                po[0:HB, nh * 5 + 2 * k:nh * 5 + 2 * k + 2],
                                lhsT=tk_red[k][:], rhs=sel[:],
                                start=True, stop=True)
                        nc.tensor.matmul(po[0:HB, nh * 5 + 4:nh * 5 + 5],
                                         lhsT=s_red[:], rhs=fusw[:],
                                         start=True, stop=True)

                    osb = wpool.tile([8, 10], F32, tag="osb")
                    nc.scalar.copy(osb[:], po[:])
                    nc.scalar.dma_start(
                        out_d.rearrange("(nh b) c -> b nh c", nh=2),
                        osb.rearrange("b (nh c) -> b nh c", nh=2))
                else:
                    out_sb = wpool.tile([16, 5], F32, tag="out_sb")
                    nc.vector.memset(out_sb[:], 0.0)
                    nc.scalar.dma_start(out_d[:], out_sb[:])

            if timing_loop is None:
                do_loads(gp_ring[0], w1t_single)
                do_mid(gp_ring[0], w1t_single, dconv_ring[0], gconv_ring[0])
                do_late(dconv_ring[0], gconv_ring[0])
            else:
                # 3-stage software pipeline over the timing loop: tick i's
                # convs/output (late) + tick i+1's dct/sobel tail (mid) +
                # tick i+2's loads all overlap, so the in-order PE stream
                # [late, mid-PE, stage-1] never stalls on the sobel chain
                # and the x DMA stream stays saturated. One tick == one
                # full kernel evaluation.
                hints = (mybir.EngineType.PE,
                         mybir.EngineType.SP,
                         mybir.EngineType.DVE,
                         mybir.EngineType.Activation)

                def stage_loads(pipe, iv):
                    gp = pipe.intermediate_tile(
                        [NCH, 8, 4, 256], F32, name="gp", prealloc=gp_ring)
                    w1t0 = pipe.intermediate_tile(
                        [128, 384], F32, name="w1t0", bufs=1,
                        prealloc=[w1t_single[0]])
                    w1t1 = pipe.intermediate_tile(
                        [128, 384], F32, name="w1t1", bufs=1,
                        prealloc=[w1t_single[1]])
                    do_loads(gp, [w1t0, w1t1])
                    return (gp, w1t0, w1t1)

                def stage_mid(pipe, iv, tl):
                    gp, w1t0, w1t1 = tl
                    dconv = pipe.intermediate_tile(
                        [3, B, 10, 10], F32R, name="dconv",
                        prealloc=dconv_ring)
                    gconv = pipe.intermediate_tile(
                        [3, B, 10, 10], F32R, name="gconv",
                        prealloc=gconv_ring)
                    do_mid(gp, [w1t0, w1t1], dconv, gconv)
                    return (dconv, gconv)

                def stage_late(pipe, iv, tl):
                    dconv, gconv = tl
                    do_late(dconv, gconv)

                unroll = int(os.environ.get("KUNROLL", "4"))
                kw = {}
                if os.environ.get("KAUTOMARK", "1") == "1" and unroll % 4 == 0:
                    kw["auto_markers"] = hints
                if os.environ.get("KHINTS", "1") == "1":
                    kw["hint_engines"] = hints
                tc.For_i_pipelined([stage_loads, stage_mid, stage_late],
                                   0, timing_loop,
                                   unroll=unroll, staged_num_bufs=RING,
                                   staggered_reset=(
                                       os.environ.get("KSTAGR", "1") == "1"),
                                   name="mainloop", **kw)

    nc.compile()
    return nc


_NC = {}


def _get_nc(timing_loop=None):
    if timing_loop not in _NC:
        _NC[timing_loop] = _build_nc(timing_loop)
    return _NC[timing_loop]


def _make_consts(conv_dct_w, conv_dct_b, bn_dct_g, bn_dct_b,
                 conv_grad_w, conv_grad_b, bn_grad_g, bn_grad_b,
                 fus_w, cls_w):
    A = build_A()
    consts = {}
    consts["at"] = np.ascontiguousarray(A.T)
    consts["ident"] = np.eye(8, dtype=np.float32)

    BN_EPS = 1e-5
    convw = np.zeros((3, 1152), np.float32)
    bias = np.zeros((128, 1), np.float32)
    for br, (w, b, g, beta) in enumerate((
            (conv_dct_w, conv_dct_b, bn_dct_g, bn_dct_b),
            (conv_grad_w, conv_grad_b, bn_grad_g, bn_grad_b))):
        g_eff = (g / np.sqrt(1.0 + BN_EPS)).astype(np.float32)
        w_eff = w * g_eff[:, None, None, None]
        if br == 1:
            w_eff = w_eff * 0.25  # fold the 4-sample average
        b_eff = b * g_eff + beta
        for di in range(3):
            for dj in range(3):
                # convw[ic, (br*9+di*3+dj)*64 + oc] = w_eff[oc, ic, di, dj]
                off = (br * 9 + di * 3 + dj) * 64
                convw[:, off:off + 64] = w_eff[:, :, di, dj].T
        bias[br * 64:(br + 1) * 64, 0] = b_eff
    consts["convw"] = convw
    consts["bias"] = bias

    ccls = np.zeros((128, 128), np.float32)
    for k in range(2):
        ccls[0:64, k * 64:(k + 1) * 64] = cls_w[k].reshape(64, 64)
        ccls[64:128, k * 64:(k + 1) * 64] = cls_w[k].reshape(64, 64)
    consts["ccls"] = ccls
    consts["fusw"] = np.ascontiguousarray(np.tile(fus_w[0][:, None] / 64.0, (2, 1)))
    sel = np.zeros((128, 2), np.float32)
    sel[0:64, 0] = 1.0
    sel[64:128, 1] = 1.0
    consts["sel"] = sel
    return consts


def kernel_with_results(x, conv_dct_w, conv_dct_b, bn_dct_g, bn_dct_b,
                        conv_grad_w, conv_grad_b, bn_grad_g, bn_grad_b,
                        fus_w, fus_b, cls_w, cls_b, trace=False):
    nc = _get_nc()
    consts = _make_consts(conv_dct_w, conv_dct_b, bn_dct_g, bn_dct_b,
                          conv_grad_w, conv_grad_b, bn_grad_g, bn_grad_b,
                          fus_w, cls_w)
    x = np.ascontiguousarray(np.asarray(x, np.float32))
    in_maps = []
    for i in range(N_CORES):
        m = {"x": np.ascontiguousarray(x[i * B:(i + 1) * B])}
        m.update(consts)
        in_maps.append(m)
    res = run_bass_kernel_spmd(nc, in_maps, list(range(N_CORES)), trace=trace)

    outs = []
    for i in range(N_CORES):
        r = res.results[i]["out"]  # [16, 5]
        Pd = r[:, [0, 2]]
        Pg = r[:, [1, 3]]
        sl = r[:, 4] + np.float32(fus_b[0])
        w = 1.0 / (1.0 + np.exp(-sl))[:, None]
        outs.append(w * Pd + (1.0 - w) * Pg + np.asarray(cls_b)[None, :])
    return np.concatenate(outs, axis=0).astype(np.float32), res


def kernel(**inputs):
    out, _ = kernel_with_results(**inputs)
    return out
